# revision 13
# baseline (speedup 1.0000x reference)
"""DeepseekV4 hash-router MoE routing kernel for Trainium2 (8 NeuronCores).

Strategy (data-parallel over tokens, per sharding hint):
  - Shard the flattened token dim N=16384 across 8 cores (2048 tokens each).
  - Ship hidden PRE-TRANSPOSED and cast to bf16 on the host in a
    [tile, d-partition, (d-block, token)] layout so the gate matmul's lhsT
    tiles stream straight out of DRAM with no PE transposes at all and at
    half the HBM bytes of fp32 (bf16 rounding contributes ~7e-4 l2 rel
    error vs the 2e-2 gate). Replicate the bf16 [D, E] transposed gate
    weight and the one-hot routing table on every core.
  - Per core: the whole [D, NLOC] bf16 hidden transpose stays resident in
    SBUF (64 KB/partition); 16 token tiles x 16 d-blocks of [128, 128] x
    [128, 256] bf16 matmuls accumulate logits in PSUM (one [128, 1024]
    PSUM tile per 4-tile group), sqrt(softplus(x)) = exp(0.5*ln(ln(e^x+1)))
    runs on the scalar engine batched [128, 1024] per group (Exp unload
    doubles as the PSUM->SBUF move; all funcs live in the single
    natural_log_exp table), the routing rows come from dma_gather of a
    one-hot [V, E] uint8 table (vocab split 4x32000(+zero row) so indices
    fit int16, parts OR-merged on u32 views), and a fused DVE
    multiply+reduce normalizes the masked scores.
  - rmap is written with ONE [128, 4096] contiguous DMA (the gathered
    one-hot doubles as the routing map); probs with one [128, 4096B] DMA
    per 4-tile group.
  - No cross-core communication; outputs are concatenated on the host.
"""

import numpy as np

import concourse.bass as bass
import concourse.mybir as mybir
import concourse.tile as tile
from concourse import bacc
from concourse.bass import IndirectOffsetOnAxis
from concourse.bass_utils import run_bass_kernel_spmd

# Problem shape (hardcoded; kernel.py must be self-contained).
B, S, D = 4, 4096, 2048
E, K, V = 256, 8, 128000
SCALE = 2.5
NCORES = 8
N = B * S            # 16384 flattened tokens
NLOC = N // NCORES   # 2048 tokens per core
P = 128              # partitions
NT = NLOC // P       # 16 token tiles per core
ND = D // P          # 16 contraction blocks
NPART = 4            # vocab split for int16 dma_gather indices
PART = 32000         # vocab rows per part (4*32000 = V)
PR = PART + 1        # +1 zero row per part for out-of-part tokens
NCHUNK = 2           # dma_gather calls per part (<=1024 descriptors per call)
CH = NLOC // NCHUNK  # idxs per dma_gather call
CCH = NT // NCHUNK   # token-tile columns per call

F32 = mybir.dt.float32
BF16 = mybir.dt.bfloat16
I32 = mybir.dt.int32
U8 = mybir.dt.uint8
U32 = mybir.dt.uint32
AF = mybir.ActivationFunctionType
OP = mybir.AluOpType

_CACHE: dict = {}


def _build(
    reps: int = 1,
    grp: int = 4,
    mm_bufs: int = 3,
    sc_bufs: int = 2,
    no_gather: bool = False,
    no_mm: bool = False,
    no_hid_dma: bool = False,
    only_gather: bool = False,
    nchunk: int = NCHUNK,
    single_packet: bool = True,
    gather_mode: str = "dma_gather",
):
    CH_ = NLOC // nchunk   # idxs per dma_gather call
    CCH_ = NT // nchunk    # token-tile columns per call
    nc = bacc.Bacc(
        "TRN2", target_bir_lowering=False, debug=False, enable_asserts=False
    )

    # hid row j*128+p, col b*128+q  ==  hidden_local[q*NT+j, b*128+p]
    hid = nc.dram_tensor("hid", [NT * P, ND * P], BF16, kind="ExternalInput")
    tids = nc.dram_tensor("tids", [NLOC], I32, kind="ExternalInput")
    wt = nc.dram_tensor("wt", [D, E], BF16, kind="ExternalInput")
    if gather_mode == "dma_gather":
        onehot = nc.dram_tensor("onehot", [NPART * PR, E], U8, kind="ExternalInput")
        idx4 = nc.dram_tensor(
            "idx4", [P, NPART * nchunk * (CH_ // 16)], mybir.dt.int16,
            kind="ExternalInput",
        )
    else:
        onehot = nc.dram_tensor("onehot", [V, E], U8, kind="ExternalInput")
    probs = nc.dram_tensor("probs", [NLOC, E], F32, kind="ExternalOutput")
    rmap = nc.dram_tensor("rmap", [NLOC, E], U8, kind="ExternalOutput")

    # Token t <-> (partition p, tile j) with t = p*NT + j (so output rows for
    # one partition are contiguous: 16 rows of 256 -> 4KB runs per DMA).
    hid_r = hid.ap().rearrange("(j p) c -> p j c", p=P)
    probs_r = probs.ap().rearrange("(p j) e -> p j e", j=NT)
    rmap_r = rmap.ap().rearrange("(p j) e -> p (j e)", j=NT)

    GRP = grp
    NG = NT // GRP

    with tile.TileContext(nc) as tc:
        with (
            tc.tile_pool(name="const", bufs=1) as cpool,
            tc.tile_pool(name="mm_ps", bufs=mm_bufs, space="PSUM") as mm_psum,
            tc.tile_pool(name="sc", bufs=sc_bufs) as sc_pool,
            tc.tile_pool(name="nrm", bufs=3) as nrm_pool,
            tc.tile_pool(name="outp", bufs=3) as out_pool,
        ):
            # Big resident tiles (allocated once, reused across reps).
            hid_sb = cpool.tile([P, NT * ND * P], BF16)
            wt_sb = cpool.tile([P, ND * E], BF16)
            oh_all = cpool.tile([P, NT * E], U8)
            tids_sb = cpool.tile([P, NT], I32)
            if gather_mode == "dma_gather":
                idx_sb = cpool.tile([P, NPART * nchunk * (CH_ // 16)], mybir.dt.int16)
                # One buffer per (chunk, part>0): the OR-merges are deferred
                # until the consuming group, so every deferred gather needs
                # its own live destination.
                gparts = [
                    cpool.tile([P, CCH_ * E], U8, name=f"gpart{i}")
                    for i in range(3 * nchunk)
                ]

            for rep in range(reps):
                # Issue the routing-row gather chain first: the idx DMA is
                # tiny and the Q7 dma_gather descgen is the longest serial
                # chain in the kernel. The OR-merges are NOT emitted here:
                # DVE is strict FIFO, so they are deferred to just before the
                # group that consumes each vocab half (avoids head-of-line
                # blocking the per-group DVE tail work).
                merge_ops: dict[int, list] = {h: [] for h in range(nchunk)}
                if gather_mode == "dma_gather" and not no_gather:
                    nc.sync.dma_start(idx_sb[:], idx4.ap())
                    IW = CH_ // 16
                    for h in range(nchunk):
                        oh_half = oh_all[:, h * CCH_ * E : (h + 1) * CCH_ * E]
                        for m in range(NPART):
                            dst = oh_half if m == 0 else gparts[3 * h + m - 1][:]
                            k = m * nchunk + h
                            nc.gpsimd.dma_gather(
                                dst.rearrange("p (c e) -> p c e", c=CCH_),
                                onehot.ap()[m * PR : (m + 1) * PR, :],
                                idx_sb[:, k * IW : (k + 1) * IW],
                                CH_,
                                CH_,
                                E,
                                single_packet=single_packet,
                            )
                            if m > 0:
                                buf = gparts[3 * h + m - 1]
                                merge_ops[h].append((oh_half, buf))

                def emit_merges(h):
                    for oh_half, buf in merge_ops[h]:
                        nc.vector.tensor_tensor(
                            out=oh_half.bitcast(U32),
                            in0=oh_half.bitcast(U32),
                            in1=buf[:].bitcast(U32),
                            op=OP.bitwise_or,
                        )
                    merge_ops[h] = []

                nc.sync.dma_start(
                    wt_sb[:].rearrange("p (b e) -> p b e", b=ND),
                    wt.ap().rearrange("(b p) e -> p b e", p=P),
                )
                tids_loaded = False
                if gather_mode == "one_call" and not no_gather:
                    nc.sync.dma_start(
                        tids_sb[:], tids.ap().rearrange("(p j) -> p j", j=NT)
                    )
                    tids_loaded = True
                    nc.gpsimd.indirect_dma_start(
                        out=oh_all[:].rearrange("p (j e) -> p j e", j=NT),
                        out_offset=None,
                        in_=onehot.ap(),
                        in_offset=IndirectOffsetOnAxis(ap=tids_sb[:, 0:NT], axis=0),
                    )

                # Hidden tiles stream in order; tile j's matmuls wait only on
                # DMA j. Two tiles are prefetched before the weight DMA so
                # the PE can start as soon as wt lands.
                def hid_dma(j):
                    if no_hid_dma:
                        return
                    nc.sync.dma_start(
                        hid_sb[:, j * ND * P : (j + 1) * ND * P], hid_r[:, j, :]
                    )

                if only_gather:
                    for h in range(nchunk):
                        emit_merges(h)
                    nc.sync.dma_start(rmap_r, oh_all[:])
                    continue

                hid_dma(0)
                hid_dma(1)

                for g in range(NG):
                    lg = mm_psum.tile([P, GRP * E], F32, tag="lg", name=f"lg_r{rep}g{g}")
                    for q in range(GRP):
                        j = g * GRP + q
                        if j + 2 < NT:
                            hid_dma(j + 2)
                        if no_mm:
                            if q == 0:
                                nc.vector.memset(lg[:], 0.5)
                            continue
                        for b in range(ND):
                            nc.tensor.matmul(
                                lg[:, q * E : (q + 1) * E],
                                lhsT=hid_sb[
                                    :, (j * ND + b) * P : (j * ND + b + 1) * P
                                ],
                                rhs=wt_sb[:, b * E : (b + 1) * E],
                                start=(b == 0),
                                stop=(b == ND - 1),
                            )

                    # scores = sqrt(softplus(x)) = exp(0.5*ln(ln(exp(x)+1))):
                    # Exp/Ln only, so every pass stays in the single
                    # natural_log_exp table. Logits are ~N(0,1): exp is safe.
                    # The Exp doubles as the PSUM->SBUF unload.
                    ex = sc_pool.tile([P, GRP * E], F32, tag="ex", name=f"ex_r{rep}g{g}")
                    nc.scalar.activation(ex[:], lg[:], AF.Exp)
                    sp = sc_pool.tile([P, GRP * E], F32, tag="sp", name=f"sp_r{rep}g{g}")
                    nc.scalar.activation(sp[:], ex[:], AF.Ln, bias=1.0)
                    lsp = sc_pool.tile([P, GRP * E], F32, tag="lsp", name=f"lsp_r{rep}g{g}")
                    nc.scalar.activation(lsp[:], sp[:], AF.Ln)
                    sc = sc_pool.tile([P, GRP * E], F32, tag="sc", name=f"sc_r{rep}g{g}")
                    nc.scalar.activation(sc[:], lsp[:], AF.Exp, scale=0.5)

                    # Routing mask for this group (gathered one-hot rows).
                    for half in range(((g + 1) * GRP - 1) // CCH_ + 1):
                        if merge_ops.get(half):
                            emit_merges(half)
                    if no_gather and gather_mode == "dma_gather" and rep == 0 and g == 0:
                        nc.vector.memset(oh_all[:], 1)

                    msc = nrm_pool.tile([P, GRP * E], F32, tag="msc", name=f"msc_r{rep}g{g}")
                    den = nrm_pool.tile([P, GRP], F32, tag="den", name=f"den_r{rep}g{g}")
                    for q in range(GRP):
                        j = g * GRP + q
                        # masked scores + their per-token sum in one DVE op
                        nc.vector.scalar_tensor_tensor(
                            out=msc[:, q * E : (q + 1) * E],
                            in0=sc[:, q * E : (q + 1) * E],
                            scalar=0.0,
                            in1=oh_all[:, j * E : (j + 1) * E],
                            op0=OP.bypass,
                            op1=OP.mult,
                            accum_out=den[:, q : q + 1],
                        )
                    rden = nrm_pool.tile([P, GRP], F32, tag="rden", name=f"rden_r{rep}g{g}")
                    nc.vector.reciprocal(rden[:], den[:])

                    probs_t = out_pool.tile(
                        [P, GRP * E], F32, tag="probs_t", name=f"pt_r{rep}g{g}"
                    )
                    for q in range(GRP):
                        nc.vector.tensor_scalar(
                            probs_t[:, q * E : (q + 1) * E],
                            msc[:, q * E : (q + 1) * E],
                            rden[:, q : q + 1],
                            SCALE,
                            op0=OP.mult,
                            op1=OP.mult,
                        )
                    nc.sync.dma_start(
                        probs_r[:, g * GRP : (g + 1) * GRP, :],
                        probs_t[:].rearrange("p (q e) -> p q e", q=GRP),
                    )

                # One contiguous 4KB-per-partition DMA writes the whole
                # routing map (the gathered one-hot IS the routing map).
                nc.sync.dma_start(rmap_r, oh_all[:])

    nc.compile()
    return nc


def _get_nc():
    if "nc" not in _CACHE:
        _CACHE["nc"] = _build(gather_mode=GATHER_MODE)
    return _CACHE["nc"]


GATHER_MODE = "dma_gather"


def prepare_in_maps(hidden, tids, weight, tid2eid, gather_mode=None, nchunk=NCHUNK):
    """hidden [N, D] f32, tids [N] i32, weight [E, D] f32, tid2eid [V, K]."""
    if gather_mode is None:
        gather_mode = GATHER_MODE
    CH_ = NLOC // nchunk
    CCH_ = NT // nchunk
    bf16 = mybir.dt.np(BF16)
    wt = np.ascontiguousarray(
        np.asarray(weight, dtype=np.float32).T.astype(bf16)
    )  # [D, E] bf16
    t2e = np.asarray(tid2eid).astype(np.int64)
    onehot = np.zeros((V, E), dtype=np.uint8)
    onehot[np.arange(V)[:, None], t2e] = 1  # [V, E] one-hot layout of tid2eid

    if gather_mode == "dma_gather":
        oh_ship = np.zeros((NPART * PR, E), dtype=np.uint8)
        for m in range(NPART):
            oh_ship[m * PR : m * PR + PART] = onehot[m * PART : (m + 1) * PART]
    else:
        oh_ship = onehot

    hidden = np.asarray(hidden, dtype=np.float32)
    in_maps = []
    for c in range(NCORES):
        tl = np.ascontiguousarray(tids[c * NLOC : (c + 1) * NLOC])
        hl = hidden[c * NLOC : (c + 1) * NLOC]
        # [token-part p, tile j, block b, d-part] -> [j, d-part, b, token-part]
        hs = (
            hl.reshape(P, NT, ND, P)
            .transpose(1, 3, 2, 0)
            .reshape(NT * P, ND * P)
            .astype(bf16)
        )
        m = {
            "hid": np.ascontiguousarray(hs),
            "tids": tl,
            "wt": wt,
            "onehot": oh_ship,
        }
        if gather_mode == "dma_gather":
            # dma_gather int16 indices, vocab split into NPART parts with a
            # zero row at local index PART for out-of-part tokens; indices
            # wrapped into 16 partitions and replicated across Q7 cores.
            tid_pc = tl.astype(np.int64).reshape(P, NT)
            cols = []
            for mm in range(NPART):
                for h in range(nchunk):
                    lin = tid_pc[:, h * CCH_ : (h + 1) * CCH_].T.ravel()  # [CH_]
                    v = lin - mm * PART
                    vm = np.where((v >= 0) & (v < PART), v, PART).astype(np.int16)
                    wrapped = vm.reshape(CH_ // 16, 16).T  # [16, CH_/16]
                    cols.append(np.tile(wrapped, (8, 1)))  # [128, CH_/16]
            m["idx4"] = np.ascontiguousarray(np.concatenate(cols, axis=1))
        in_maps.append(m)
    return in_maps


def kernel(hidden, token_ids, weight, tid2eid):
    hidden = np.asarray(hidden, dtype=np.float32).reshape(N, D)
    tids = np.asarray(token_ids).reshape(N).astype(np.int32)

    nc = _get_nc()
    in_maps = prepare_in_maps(hidden, tids, weight, tid2eid)
    res = run_bass_kernel_spmd(nc, in_maps, core_ids=list(range(NCORES)))
    _CACHE["last_results"] = res

    probs = np.concatenate([r["probs"] for r in res.results], axis=0)
    rmap = np.concatenate([r["rmap"] for r in res.results], axis=0)
    return probs, rmap.astype(bool)
